# revision 28
# baseline (speedup 1.0000x reference)
"""2-layer GCN (GCNConv -> ReLU -> GCNConv -> Sigmoid) on 8 TRN2 NeuronCores.

Strategy (dst-node sharding, 8 cores):
  - Nodes sharded by destination range: core c owns dst rows [c*NPC, (c+1)*NPC).
  - Fold the symmetric normalization into per-node scales:
        out_d = sigmoid(dinv_d * (A0 @ (dinv*relu(dinv*(A0 @ (dinv*x@W1)) ...)))...
    so the sparse aggregation A0 (unweighted multi-adjacency + self loops)
    acts on 50-wide "scaled" tables and no per-edge weight is needed.
  - Self-loops are NOT gathered: the identity part of A0 is added locally in
    the epilogues from SBUF-resident z / h1 tables.  This also equalizes the
    per-(core,range) edge segments so the uniform chunk grid has ~3% padding.
  - Per layer: z table (node-major, bf16, rows padded to 256B) is AllGathered;
    each core gathers z[src] for its edges with dma_gather in PREPARE_ONLY +
    trigger_dma mode on a single SWDGE queue.  The Q7 descriptor generation
    (~8.5us per 1024-token batch) is the kernel's critical path; prep/trigger
    keeps the DMA drains, PE work and everything else underneath it.
  - Chunks of <=128 edges spanning <32 dst nodes are reduced with one-hot S1
    matmuls (S1 built on-device by DVE is_equal vs iota) into per-region
    partial-sum tiles kept in SBUF, split by chunk parity so a dst straddling
    two consecutive chunks never collides inside one region.
  - The partial regions are merged into per-dst-block PSUM accumulators with
    a second round of one-hot matmuls (slot->dst-row one-hots built by DVE
    from a host slot-map table; each (block, region) merge scans a fixed
    MK=3-tile candidate window around the expected slot position, verified
    host-side).  No scatter-add DMAs at all.
  - Epilogues run fused per dst block straight out of PSUM: self-loop add,
    dinv scales, relu / transpose + W2 matmul + sigmoid.

Synchronization: Tile's prepare_only consumer sync is broken (it pre-bumps
the DMASW lane sems, so auto-generated waits pass before the DMA lands), and
a single shared completion sem cannot prove an individual batch drained (the
16 SDMA engines increment it independently).  Sound scheme used here: all
SWDGE work on one queue (ring FIFO = pool order, pinned with no-sync dep
edges), grouped into windows of 2 with the completion sem alternating
between two sems; after window i+1's generation a pool barrier waits
sem[i%2] >= 16*(all instructions ever assigned to that sem) -- nothing newer
on that sem is in flight, so this proves full drain -- then bumps a fence
sem consumers wait on.  Ring occupancy stays <= 2 windows = 4096 descriptors
= the carveout from dynamic_dma_scratch_size=65536.

Host side does only index/metadata preprocessing (sorting edges, degree
counts, chunk layout, slot maps) and input re-layout (x transposed + bf16).

v2 changes vs the first working version:
  - Explicit trigger counts (count=WIN) gated on a prep-completion sem
    replace count=None triggers, eliminating Tile's per-prep
    InstIncSwdgeSem (~1.4us of pool-engine time each, ~560us total).
  - The z / h1 tables and their AllGathers are split in half ("half-major"
    global table layout) so each half's collective overlaps the compute
    that produces the other half.
  - x is shipped as per-dst-block contiguous [NB, 128, KT*128] tiles
    (3KB/partition DMA lines instead of 256B).
  - DRAM zero-fill removed (unwritten table bytes are never consumed).
"""

import os
import numpy as np
import ml_dtypes

import concourse.bass as bass
import concourse.bacc as bacc
import concourse.tile as tile
from concourse.tile_rust import add_dep_helper
import concourse.mybir as mybir
from concourse.bass_utils import run_bass_kernel_spmd

BF16 = mybir.dt.bfloat16
FP16 = mybir.dt.float16
F32 = mybir.dt.float32
I16 = mybir.dt.int16

C = 8        # cores
P = 128      # partitions
SLOT = 32    # dst slots per chunk (chunk spans < 32 dst nodes)
DEAD = SLOT  # col_rel value marking a dead (padded) edge


def _merge_ct(b, tpr, npcp):
    # expected tile of dst block b's slots in a range region (slot ~ rho*dst)
    return int((b * 128 + 64) * (tpr * 128) / npcp) // 128


def _merge_t0(b, tpr, npcp, k, off):
    # host-measured slot-tile offsets vs the center tile lie in
    # [-off, -off+k-1]
    return min(max(_merge_ct(b, tpr, npcp) - off, 0), max(tpr - k, 0))


def _cfg_for(n_nodes, fin, hid, out_dim, ch_r, gb, mk):
    npc = n_nodes // C
    nb = -(-npc // P)
    npcp = nb * P
    kt = -(-fin // P)
    cfg = dict(
        N=n_nodes, FIN=fin, HID=hid, OUT=out_dim,
        NPC=npc, NB=nb, NPCP=npcp, KT=kt, KP=kt * P,
        RN=2 * npcp,                  # rows per gather range (2 cores)
        TBL=C * npcp,                 # allgathered table rows
        CH_R=ch_r,                    # chunks per (core, range), uniform
        GB=gb,                        # gather batch tokens
        BR=(ch_r * P) // gb,          # gather batches per range
        TPR=ch_r // 4,                # partial-sum tiles per range region
        MK=mk,                        # merge candidate tiles per (block, region)
    )
    assert cfg["BR"] * gb == ch_r * P and ch_r % 8 == 0 and gb % 128 == 0
    return cfg


# ----------------------------------------------------------------- host prep

def _preprocess(x, edge_index, W1, b1, W2, b2):
    N, FIN = x.shape
    HID = W1.shape[1]
    OUT = W2.shape[1]
    assert N % C == 0
    NPC = N // C
    NB = -(-NPC // P)
    NPCP = NB * P
    RN = 2 * NPCP

    rows = edge_index[0].astype(np.int64)
    cols = edge_index[1].astype(np.int64)

    # degree includes the self loop (reference adds it before segment_sum)
    deg = (np.bincount(cols, minlength=N) + 1).astype(np.float32)
    dinv = (1.0 / np.sqrt(deg.astype(np.float64))).astype(np.float32)

    # table row of node n in the allgathered table.  Layout is HALF-major:
    # [A-halves of cores 0..7 | B-halves of cores 0..7], where a core's
    # A-half is its first HA(=NBA*P) padded-local rows.  Each AllGather
    # half is then a plain contiguous collective, and each of the 4 gather
    # ranges (a half of a core quad) stays a contiguous RN-row region.
    NBA = (NPCP // P) // 2
    HA = NBA * P
    HB = NPCP - HA
    src_c = rows // NPC
    src_i = rows % NPC
    tbl_row = np.where(
        src_i < HA,
        src_c * HA + src_i,
        C * HA + src_c * HB + (src_i - HA),
    )
    src_range = tbl_row // RN
    idx_local = (tbl_row - src_range * RN).astype(np.int64)
    core = cols // NPC
    col_local = (cols - core * NPC).astype(np.int64)

    order = np.lexsort((col_local, src_range, core))
    core_s = core[order]
    rng_s = src_range[order]
    coll_s = col_local[order]
    idxl_s = idx_local[order]

    # chunk every (core, range) segment: break at 128 tokens or dst span 32
    bounds_all = {}
    max_chunks = 0
    seg_edges = {}
    for c in range(C):
        c_end = np.searchsorted(core_s, c + 1)
        c_start = np.searchsorted(core_s, c)
        for r in range(4):
            s0 = c_start + np.searchsorted(rng_s[c_start:c_end], r)
            s1 = c_start + np.searchsorted(rng_s[c_start:c_end], r + 1)
            seg_edges[(c, r)] = (s0, s1)
            cseg = coll_s[s0:s1]
            bounds = []
            i = 0
            n = len(cseg)
            while i < n:
                j = int(np.searchsorted(cseg, cseg[i] + SLOT, side="left"))
                j = min(j, i + P, n)
                bounds.append((i, j))
                i = j
            bounds_all[(c, r)] = bounds
            max_chunks = max(max_chunks, len(bounds))
    ch_r = max(8, ((max_chunks + 7) // 8) * 8)
    # dma_gather is limited to 1024 indices per instruction (SWDGE
    # descriptor-ring capacity).
    gb = 1024

    # adaptive merge window: measure, over every (core, range, chunk), how
    # far live slots' tiles stray from their dst block's center tile
    tpr0 = ch_r // 4
    dmin, dmax = 0, 0
    for (c, r), bounds in bounds_all.items():
        s0, s1 = seg_edges[(c, r)]
        cseg = coll_s[s0:s1]
        for j, (s, e) in enumerate(bounds):
            cr = cseg[s:e] - cseg[s]
            tt = (SLOT * j + cr) // P
            ct = ((cseg[s] + cr) // P * P + 64) * (tpr0 * P) // NPCP // P
            d = tt - ct
            dmin = min(dmin, int(d.min()))
            dmax = max(dmax, int(d.max()))
    OFF = -dmin
    MK = dmax - dmin + 1
    cfg = _cfg_for(N, FIN, HID, OUT, ch_r, gb, MK)
    cfg["OFF"] = OFF
    CH_R, GB, BR, TPR = cfg["CH_R"], cfg["GB"], cfg["BR"], cfg["TPR"]
    CHUNKS = 4 * CH_R
    DUMMY = NPCP  # dummy dst row for dead slots (never matches a block row)

    # weights / tables, shared across cores
    KP = cfg["KP"]
    KT = KP // P
    xt = np.zeros((KP, C * NPCP), dtype=ml_dtypes.bfloat16)
    xtf = np.ascontiguousarray(x.T).astype(ml_dtypes.bfloat16)
    for c in range(C):
        xt[:FIN, c * NPCP:c * NPCP + NPC] = xtf[:, c * NPC:(c + 1) * NPC]
    w1 = np.zeros((KP, 64), dtype=ml_dtypes.bfloat16)
    w1[:FIN, :HID] = W1.astype(ml_dtypes.bfloat16)
    w2 = np.zeros((64, OUT), dtype=ml_dtypes.bfloat16)
    w2[:HID, :] = W2.astype(ml_dtypes.bfloat16)
    iota32 = np.tile(np.arange(SLOT, dtype=np.float32), (P, 1)).astype(ml_dtypes.bfloat16)
    iota128 = np.tile(np.arange(P, dtype=np.float32), (P, 1))
    ident = np.eye(P, dtype=np.float32).astype(ml_dtypes.bfloat16)
    b1r = np.zeros((1, 64), np.float32)
    b1r[0, :HID] = b1
    b2r = b2.reshape(1, OUT).astype(np.float32)
    has_b1 = bool(np.any(b1))
    has_b2 = bool(np.any(b2))

    in_maps = []
    for c in range(C):
        gidx = np.zeros((4 * BR, P, GB // 16), np.int16)
        colrel_tile = np.full((P, CHUNKS), float(DEAD), np.float32)
        mrel = np.full((P, NB * 4 * MK), float(2 * P), np.float32)

        for r in range(4):
            s0, s1 = seg_edges[(c, r)]
            cseg = coll_s[s0:s1]
            iseg = idxl_s[s0:s1]
            bounds = bounds_all[(c, r)]
            gtok = np.zeros((CH_R, P), np.int64)
            crel = np.full((CH_R, P), DEAD, np.int64)
            sreg = np.full(TPR * P, DUMMY, np.int64)
            for j, (s, e) in enumerate(bounds):
                L = e - s
                # sort the chunk's tokens by src table row (HBM locality);
                # crel follows the same permutation so S1 still matches.
                perm = np.argsort(iseg[s:e], kind="stable")
                gtok[j, :L] = iseg[s:e][perm]
                if L < P:
                    gtok[j, L:] = gtok[j, L - 1]
                cr_sorted = (cseg[s:e] - cseg[s])[perm]
                crel[j, :L] = cr_sorted
                cr = cseg[s:e] - cseg[s]
                # slot map: chunk j occupies slots [32j, 32j+32)
                sreg[SLOT * j:SLOT * (j + 1)][cr] = cseg[s] + cr
            # assemble per-core tensors
            colrel_tile[:, r * CH_R:(r + 1) * CH_R] = crel.T
            for bi in range(BR):
                toks = gtok[bi * (GB // P):(bi + 1) * (GB // P)].reshape(-1)
                gidx[r * BR + bi] = np.tile(
                    toks.reshape(GB // 16, 16).T, (8, 1))
            # merge one-hot columns for this range region; coverage check:
            # every live slot's tile must be inside its block's window
            live = sreg != DUMMY
            if live.any():
                sl = np.nonzero(live)[0]
                bb = sreg[sl] // 128
                tt = sl // 128
                t0s = np.array([_merge_t0(b, TPR, NPCP, MK, OFF) for b in bb])
                assert ((tt >= t0s) & (tt < t0s + MK)).all(), \
                    "merge window MK too small"
            for b in range(NB):
                t0 = _merge_t0(b, TPR, NPCP, MK, OFF)
                for k in range(MK):
                    col = b * (4 * MK) + r * MK + k
                    seg = sreg[(t0 + k) * P:(t0 + k + 1) * P]
                    mrel[:, col] = seg - 128 * b

        nb = cfg["NB"]
        dloc = np.ones(NPCP, np.float32)
        dloc[:NPC] = dinv[c * NPC:(c + 1) * NPC]
        dinv_pp = dloc.reshape(nb, P).T.copy()          # [128, NB]
        dinv2_pp = (dloc * dloc).reshape(nb, P).T.copy()
        sqdloc = np.ones(NPCP, np.float32)
        sqdloc[:NPC] = np.sqrt(deg[c * NPC:(c + 1) * NPC])

        # xtb[b, p, k, n]: per dst-block contiguous (3KB/partition DMAs)
        xtb = np.ascontiguousarray(
            xt[:, c * NPCP:(c + 1) * NPCP]
            .reshape(KT, P, NB, P).transpose(2, 1, 0, 3)
            .reshape(NB, P, KT * P))

        m = {
            "xtb": xtb,
            "w1": w1, "w2": w2, "iota32": iota32, "iota128": iota128,
            "ident": ident,
            "colrel": colrel_tile.astype(ml_dtypes.bfloat16),
            "gidx": gidx.astype(np.int16),
            "mrel": mrel,
            "dinv_pp": dinv_pp, "dinv2_pp": dinv2_pp,
            "b1f": np.tile(b1r, (P, 1)), "b2r": b2r,
            "sqd": sqdloc.reshape(1, NPCP),
            "sqd_pp": sqdloc.reshape(nb, P).T.copy(),
        }
        in_maps.append(m)

    cfg["HAS_B1"] = has_b1
    cfg["HAS_B2"] = has_b2
    return cfg, in_maps


# ------------------------------------------------------------- program build

def _build_program(cfg, phases="full"):
    NB, KT, NPCP, RN, TBL = cfg["NB"], cfg["KT"], cfg["NPCP"], cfg["RN"], cfg["TBL"]
    CH_R, GB, BR, TPR = cfg["CH_R"], cfg["GB"], cfg["BR"], cfg["TPR"]
    OUT, MK, OFF = cfg["OUT"], cfg["MK"], cfg["OFF"]
    CHUNKS = 4 * CH_R
    SPB = GB // P      # chunk slots per gather batch
    NPAIR = SPB // 8   # psum-tile pairs per batch

    nc = bacc.Bacc("TRN2", target_bir_lowering=False, debug=False,
                   num_devices=C, num_swdge_queues=1,
                   dynamic_dma_scratch_size=65536)

    xt_d = nc.dram_tensor("xtb", [NB, P, KT * P], BF16, kind="ExternalInput")
    w1_d = nc.dram_tensor("w1", [cfg["KP"], 64], BF16, kind="ExternalInput")
    w2_d = nc.dram_tensor("w2", [64, OUT], BF16, kind="ExternalInput")
    iota_d = nc.dram_tensor("iota32", [P, SLOT], BF16, kind="ExternalInput")
    iota128_d = nc.dram_tensor("iota128", [P, P], F32, kind="ExternalInput")
    ident_d = nc.dram_tensor("ident", [P, P], BF16, kind="ExternalInput")
    colrel_d = nc.dram_tensor("colrel", [P, CHUNKS], BF16, kind="ExternalInput")
    gidx_d = nc.dram_tensor("gidx", [4 * BR, P, GB // 16], I16, kind="ExternalInput")
    mrel_d = nc.dram_tensor("mrel", [P, NB * 4 * MK], F32, kind="ExternalInput")
    dinv_d = nc.dram_tensor("dinv_pp", [P, NB], F32, kind="ExternalInput")
    dinv2_d = nc.dram_tensor("dinv2_pp", [P, NB], F32, kind="ExternalInput")
    b1f_d = nc.dram_tensor("b1f", [P, 64], F32, kind="ExternalInput")
    b2_d = nc.dram_tensor("b2r", [1, OUT], F32, kind="ExternalInput")
    sqd_d = nc.dram_tensor("sqd", [1, NPCP], F32, kind="ExternalInput")
    sqdpp_d = nc.dram_tensor("sqd_pp", [P, NB], F32, kind="ExternalInput")
    out_d = nc.dram_tensor("out", [NPCP, OUT], F32, kind="ExternalOutput")
    dbg_d = None
    if phases != "full":
        dbg_d = nc.dram_tensor("dbg", [TBL, P], F32, kind="ExternalOutput")

    # ---- SWDGE window/fence machinery (see module docstring) -------------
    WIN = 2
    wsems = [nc.alloc_semaphore("sww0"), nc.alloc_semaphore("sww1")]
    fence_sem = nc.alloc_semaphore("swfence")
    sw = {
        "counts": [0, 0], "wid": 0, "items": 0, "pending": [],
        "last": None, "fences": 0, "gwids": [],
    }

    def _chain(bi):
        if sw["last"] is not None:
            add_dep_helper(bi.ins, sw["last"], sync=False,
                           reason="swdge ring order")
        sw["last"] = bi.ins
        return bi

    def _emit_barrier():
        wid, sidx, cnt = sw["pending"].pop(0)
        _chain(nc.gpsimd.wait_ge(wsems[sidx], 16 * cnt))
        _chain(nc.gpsimd.sem_inc(fence_sem, 1))
        sw["fences"] += 1
        assert sw["fences"] == wid + 1

    def _close_window():
        if sw["items"] == 0:
            return
        sidx = sw["wid"] % 2
        sw["pending"].append((sw["wid"], sidx, sw["counts"][sidx]))
        sw["wid"] += 1
        sw["items"] = 0

    def sw_sem():
        return wsems[sw["wid"] % 2]

    FIRE_N = 4  # batches per coalesced count=None trigger

    def sw_track(prep):
        wid = sw["wid"]
        sw["counts"][wid % 2] += 1
        sw["items"] += 1
        sw["unfired"] = sw.get("unfired", 0) + 1
        _chain(prep)
        if sw["unfired"] == FIRE_N:
            _chain(nc.gpsimd.trigger_dma(count=None, queue_num=0))
            sw["unfired"] = 0
        if sw["items"] == WIN:
            _close_window()
            while len(sw["pending"]) >= 2:
                _emit_barrier()
        return wid

    def sw_flush():
        if sw.get("unfired", 0):
            _chain(nc.gpsimd.trigger_dma(count=None, queue_num=0))
            sw["unfired"] = 0
        _close_window()
        while sw["pending"]:
            _emit_barrier()
        return sw["fences"]

    GIXT_BUFS = 6

    with tile.TileContext(nc) as tc:
        with tc.tile_pool(name="const", bufs=1) as cpool, \
             tc.tile_pool(name="sb", bufs=2) as sb, \
             tc.tile_pool(name="msgp", bufs=4) as msgp, \
             tc.tile_pool(name="parts", bufs=4) as parts_pool, \
             tc.tile_pool(name="psum", bufs=2, space="PSUM") as pp, \
             tc.tile_pool(name="dram", bufs=1, space="DRAM") as dram:

            # ---- constants
            w1_t = cpool.tile([P, KT, 64], BF16)
            nc.sync.dma_start(
                out=w1_t[:], in_=w1_d.ap().rearrange("(k p) e -> p k e", p=P))
            w2_t = cpool.tile([64, OUT], BF16)
            nc.sync.dma_start(out=w2_t[:], in_=w2_d.ap())
            iota_t = cpool.tile([P, SLOT], BF16)
            nc.sync.dma_start(out=iota_t[:], in_=iota_d.ap())
            iota128_t = cpool.tile([P, P], F32)
            nc.sync.dma_start(out=iota128_t[:], in_=iota128_d.ap())
            ident_t = cpool.tile([P, P], BF16)
            nc.sync.dma_start(out=ident_t[:], in_=ident_d.ap())
            colrel_t = cpool.tile([P, CHUNKS], BF16)
            nc.sync.dma_start(out=colrel_t[:], in_=colrel_d.ap())
            mrel_t = cpool.tile([P, NB * 4 * MK], F32)
            nc.sync.dma_start(out=mrel_t[:], in_=mrel_d.ap())
            dinv_t = cpool.tile([P, NB], F32)
            nc.sync.dma_start(out=dinv_t[:], in_=dinv_d.ap())
            dinv2_t = cpool.tile([P, NB], F32)
            nc.sync.dma_start(out=dinv2_t[:], in_=dinv2_d.ap())
            b2_t = cpool.tile([1, OUT], F32)
            nc.sync.dma_start(out=b2_t[:], in_=b2_d.ap())
            sqd_t = None
            if cfg["HAS_B2"]:
                sqd_t = cpool.tile([1, NPCP], F32)
                nc.sync.dma_start(out=sqd_t[:], in_=sqd_d.ap())
            b1f_t = None
            sqdpp_t = None
            if cfg["HAS_B1"]:
                b1f_t = cpool.tile([P, 64], F32)
                nc.sync.dma_start(out=b1f_t[:], in_=b1f_d.ap())
                sqdpp_t = cpool.tile([P, NB], F32)
                nc.sync.dma_start(out=sqdpp_t[:], in_=sqdpp_d.ap())

            # SBUF-resident local scaled tables (self-loop add in epilogues)
            z_sb = cpool.tile([P, NB, 64], BF16)
            h1_sb = cpool.tile([P, NB, 64], BF16)



            # ---- DRAM temporaries.  The per-core local tables are split
            # into two half tiles so each half's AllGather can launch as
            # soon as that half's blocks are written (overlapping the
            # collective with remaining compute).
            NBA = NB // 2
            HA, HB = NBA * P, NPCP - NBA * P
            z_locA = dram.tile([HA, P], BF16, name="z_locA")
            z_locB = dram.tile([HB, P], BF16, name="z_locB")
            zfullA = dram.tile([C * HA, P], BF16, name="zfullA")
            zfullB = dram.tile([C * HB, P], BF16, name="zfullB")
            h1_locA = dram.tile([HA, P], BF16, name="h1_locA")
            h1_locB = dram.tile([HB, P], BF16, name="h1_locB")
            h1fullA = dram.tile([C * HA, P], BF16, name="h1fullA")
            h1fullB = dram.tile([C * HB, P], BF16, name="h1fullB")

            def loc_row(t_a, t_b, b):
                if b < NBA:
                    return t_a[:][b * P:(b + 1) * P, 0:64]
                return t_b[:][(b - NBA) * P:(b - NBA) * P + P, 0:64]

            def ag_half(loc, full):
                # half-major table: ranges 0,1 live in the A tensor,
                # ranges 2,3 in the B tensor; each AllGather is a plain
                # whole-tile contiguous collective.
                nc.gpsimd.collective_compute(
                    "AllGather", mybir.AluOpType.bypass,
                    replica_groups=[list(range(C))],
                    ins=[loc.opt()], outs=[full.opt()])

            # ---- phase Z: z_loc = dinv * (x @ W1), node-major bf16
            for b in range(NB):
                xtt = sb.tile([P, KT, P], BF16, tag="xtt")
                nc.sync.dma_start(out=xtt[:], in_=xt_d.ap()[b])
                psz = pp.tile([P, 64], F32, tag="psz")
                for k in range(KT):
                    nc.tensor.matmul(
                        out=psz[:], lhsT=xtt[:, k, :], rhs=w1_t[:, k, :],
                        start=(k == 0), stop=(k == KT - 1))
                nc.scalar.activation(
                    out=z_sb[:, b, :], in_=psz[:],
                    func=mybir.ActivationFunctionType.Copy,
                    scale=dinv_t[:, b:b + 1])
                nc.sync.dma_start(
                    out=loc_row(z_locA, z_locB, b), in_=z_sb[:, b, :])
                if b == NBA - 1:
                    ag_half(z_locA, zfullA)
            ag_half(z_locB, zfullB)

            def dump_dbg(src, rows, row_off=0):
                nc.gpsimd.dma_start(
                    out=dbg_d.ap()[row_off:row_off + rows, :], in_=src[:][:rows, :])

            # ---- aggregation layer: bi-major gathers + chunk-reduce into
            # 4 range regions [P, TPR, 64] fp16, with per-dst-block merge
            # matmuls + the layer epilogue STREAMING behind the gather rows
            # (block b is merged once tiles t0(b)..t0(b)+MK-1 of every
            # region are complete, i.e. after row (t0(b)+MK)/2 - 1).
            def merge_block(b, partials, epilogue_fn):
                t0 = _merge_t0(b, TPR, NPCP, MK, OFF)
                oh = sb.tile([P, 4 * MK, P], FP16, tag="oh")
                c0 = b * (4 * MK)
                nc.vector.tensor_tensor(
                    out=oh[:],
                    in0=iota128_t[:][:, None, :].to_broadcast([P, 4 * MK, P]),
                    in1=mrel_t[:, c0:c0 + 4 * MK][:, :, None].to_broadcast(
                        [P, 4 * MK, P]),
                    op=mybir.AluOpType.is_equal)
                psm = pp.tile([P, 64], F32, tag="mg")
                i = 0
                n_mm = 4 * MK
                for r in range(4):
                    for k in range(MK):
                        nc.tensor.matmul(
                            out=psm[:], lhsT=oh[:, r * MK + k, :],
                            rhs=partials[r][:, t0 + k, :],
                            start=(i == 0), stop=(i == n_mm - 1),
                            skip_group_check=True)
                        i += 1
                epilogue_fn(b, psm)

            def agg_layer(src_full, epilogue_fn):
                fullA, fullB = src_full
                partials = [parts_pool.tile([P, TPR, 64], FP16, tag="parts",
                                             name=f"parts_r{r}")
                            for r in range(4)]
                next_b = [0]

                def drain_ready(tiles_done):
                    while next_b[0] < NB and (
                            _merge_t0(next_b[0], TPR, NPCP, MK, OFF) + MK
                            <= tiles_done):
                        merge_block(next_b[0], partials, epilogue_fn)
                        next_b[0] += 1

                for bi in range(BR):
                    for r in range(4):
                        bidx = r * BR + bi
                        g = len(sw["gwids"])
                        gixt = sb.tile([P, GB // 16], I16, tag="gixt",
                                       bufs=GIXT_BUFS)
                        gl = nc.sync.dma_start(out=gixt[:], in_=gidx_d.ap()[bidx])
                        if g >= GIXT_BUFS:
                            # deferred gather reads its idx tile when the
                            # trigger fires; the fence covering the buffer's
                            # previous user implies that trigger fired
                            gl._wait_ge(fence_sem, sw["gwids"][g - GIXT_BUFS] + 1)
                        msg = msgp.tile([P, SPB, P], BF16, tag="msg",
                                        bufs=8)
                        src_t = fullA if r < 2 else fullB
                        rr = r % 2
                        prep = nc.gpsimd.dma_gather(
                            out_ap=msg[:],
                            in_ap=src_t[:][rr * RN:(rr + 1) * RN, :],
                            idxs_ap=gixt[:],
                            num_idxs=GB, num_idxs_reg=GB, elem_size=P,
                            prepare_only=True, sem=sw_sem(), queue_num=0)
                        gwid = sw_track(prep)
                        sw["gwids"].append(gwid)
                        s1t = sb.tile([P, SPB, SLOT], BF16, tag="s1t")
                        cb = bidx * SPB
                        nc.vector.tensor_tensor(
                            out=s1t[:],
                            in0=iota_t[:][:, None, :].to_broadcast([P, SPB, SLOT]),
                            in1=colrel_t[:, cb:cb + SPB][:, :, None].to_broadcast(
                                [P, SPB, SLOT]),
                            op=mybir.AluOpType.is_equal)
                        for pair in range(NPAIR):
                            ps_e = pp.tile([P, 64], F32, tag="pse")
                            ps_o = pp.tile([P, 64], F32, tag="pso")
                            ps = [ps_e, ps_o]
                            for jj in range(8):
                                cl = pair * 8 + jj
                                q4 = jj % 4
                                mm = nc.tensor.matmul(
                                    out=ps[jj // 4][SLOT * q4:SLOT * (q4 + 1), :],
                                    lhsT=s1t[:, cl, :],
                                    rhs=msg[:, cl, 0:64],
                                    start=True, stop=True,
                                    tile_position=(0, SLOT * q4),
                                    skip_group_check=True)
                                mm._wait_ge(fence_sem, gwid + 1)
                            trb = (bi * NPAIR + pair) * 2
                            nc.vector.tensor_copy(
                                out=partials[r][:, trb, :], in_=ps[0][:])
                            nc.scalar.copy(
                                out=partials[r][:, trb + 1, :], in_=ps[1][:])
                    drain_ready(2 * (bi + 1) - 2)
                sw_flush()
                drain_ready(TPR)
                assert next_b[0] == NB

            order = ["z", "l1", "full"]
            lvl = order.index(phases) if phases in order else 2

            # L1 epilogue: h1 = relu(dinv^2*(agg + z_self) [+ dinv*b1])
            def epilogue1(b, psm):
                st = sb.tile([P, 64], F32, tag="st")
                nc.vector.tensor_tensor(
                    out=st[:], in0=psm[:], in1=z_sb[:, b, :],
                    op=mybir.AluOpType.add)
                if cfg["HAS_B1"]:
                    tmp = sb.tile([P, 64], F32, tag="tmpb")
                    nc.vector.tensor_scalar_mul(
                        tmp[:], b1f_t[:], sqdpp_t[:, b:b + 1])
                    nc.vector.tensor_tensor(
                        out=st[:], in0=st[:], in1=tmp[:], op=mybir.AluOpType.add)
                nc.scalar.activation(
                    out=h1_sb[:, b, :], in_=st[:],
                    func=mybir.ActivationFunctionType.Relu,
                    scale=dinv2_t[:, b:b + 1])
                nc.sync.dma_start(
                    out=loc_row(h1_locA, h1_locB, b), in_=h1_sb[:, b, :])
                if b == NBA - 1:
                    ag_half(h1_locA, h1fullA)

            # L2 epilogue: out = sigmoid(dinv * ((agg2 + h1_self) @ W2) [+ b2])
            def epilogue2(b, psm):
                st = sb.tile([P, 64], BF16, tag="st2")
                nc.vector.tensor_tensor(
                    out=st[:], in0=psm[:], in1=h1_sb[:, b, :],
                    op=mybir.AluOpType.add)
                tp = pp.tile([64, P], BF16, tag="psz")
                nc.tensor.transpose(out=tp[:], in_=st[:], identity=ident_t[:])
                zt = sb.tile([64, P], BF16, tag="zt")
                nc.scalar.copy(out=zt[:], in_=tp[:])
                ps3 = pp.tile([P, 64], F32, tag="pso")
                nc.tensor.matmul(
                    out=ps3[:, 0:OUT], lhsT=zt[:], rhs=w2_t[:],
                    start=True, stop=not cfg["HAS_B2"],
                    skip_group_check=True)
                if cfg["HAS_B2"]:
                    nc.tensor.matmul(
                        out=ps3[:, 0:OUT], lhsT=sqd_t[:, b * P:(b + 1) * P],
                        rhs=b2_t[:], start=False, stop=True,
                        skip_group_check=True)
                ot = sb.tile([P, OUT], F32, tag="ot")
                nc.scalar.activation(
                    out=ot[:], in_=ps3[:, 0:OUT],
                    func=mybir.ActivationFunctionType.Sigmoid,
                    scale=dinv_t[:, b:b + 1])
                nc.sync.dma_start(
                    out=out_d.ap()[b * P:(b + 1) * P, :], in_=ot[:])

            # ================= layer 1 =================
            if lvl >= 1:
                agg_layer((zfullA, zfullB), epilogue1)
            if phases == "z":
                dump_dbg(zfullA, C * HA)
                dump_dbg(zfullB, C * HB, row_off=C * HA)
            if phases == "l1":
                dump_dbg(h1_locA, HA)
                dump_dbg(h1_locB, HB, row_off=HA)

            if lvl >= 2:
                ag_half(h1_locB, h1fullB)

                # ================= layer 2 =================
                agg_layer((h1fullA, h1fullB), epilogue2)

    nc.compile()
    return nc


_PROGRAM_CACHE = {}
LAST_EXEC_NS = None
LAST_TRACE = None


def _get_program(cfg):
    key = tuple(sorted((k, v) for k, v in cfg.items()))
    if key not in _PROGRAM_CACHE:
        _PROGRAM_CACHE[key] = _build_program(cfg)
    return _PROGRAM_CACHE[key]


def kernel(x, edge_index, W1, b1, W2, b2):
    x = np.asarray(x, np.float32)
    edge_index = np.asarray(edge_index)
    W1 = np.asarray(W1, np.float32)
    b1 = np.asarray(b1, np.float32)
    W2 = np.asarray(W2, np.float32)
    b2 = np.asarray(b2, np.float32)

    cfg, in_maps = _preprocess(x, edge_index, W1, b1, W2, b2)
    nc = _get_program(cfg)
    trace = bool(os.environ.get("KERNEL_TRACE"))
    res = run_bass_kernel_spmd(nc, in_maps, core_ids=list(range(C)), trace=trace)
    global LAST_EXEC_NS, LAST_TRACE
    if res.exec_time_ns:
        LAST_EXEC_NS = res.exec_time_ns
        LAST_TRACE = res
    NPC, OUT = cfg["NPC"], cfg["OUT"]
    out = np.empty((cfg["N"], OUT), np.float32)
    for c in range(C):
        out[c * NPC:(c + 1) * NPC] = res.results[c]["out"][:NPC]
    return out



# revision 32
# speedup vs baseline: 1.1379x; 1.1379x over previous
"""2-layer GCN (GCNConv -> ReLU -> GCNConv -> Sigmoid) on 8 TRN2 NeuronCores.

Strategy (dst-node sharding, 8 cores):
  - Nodes sharded by destination range: core c owns dst rows [c*NPC, (c+1)*NPC).
  - Fold the symmetric normalization into per-node scales:
        out_d = sigmoid(dinv_d * (A0 @ (dinv*relu(dinv*(A0 @ (dinv*x@W1)) ...)))...
    so the sparse aggregation A0 (unweighted multi-adjacency + self loops)
    acts on 50-wide "scaled" tables and no per-edge weight is needed.
  - Self-loops are NOT gathered: the identity part of A0 is added locally in
    the epilogues from SBUF-resident z / h1 tables.  This also equalizes the
    per-(core,range) edge segments so the uniform chunk grid has ~3% padding.
  - Per layer: z table (node-major, bf16, rows padded to 256B) is AllGathered;
    each core gathers z[src] for its edges with dma_gather in PREPARE_ONLY +
    trigger_dma mode on a single SWDGE queue.  The Q7 descriptor generation
    (~8.5us per 1024-token batch) is the kernel's critical path; prep/trigger
    keeps the DMA drains, PE work and everything else underneath it.
  - Chunks of <=128 edges spanning <32 dst nodes are reduced with one-hot S1
    matmuls (S1 built on-device by DVE is_equal vs iota) into per-region
    partial-sum tiles kept in SBUF, split by chunk parity so a dst straddling
    two consecutive chunks never collides inside one region.
  - The partial regions are merged into per-dst-block PSUM accumulators with
    a second round of one-hot matmuls (slot->dst-row one-hots built by DVE
    from a host slot-map table; each (block, region) merge scans a fixed
    MK=3-tile candidate window around the expected slot position, verified
    host-side).  No scatter-add DMAs at all.
  - Epilogues run fused per dst block straight out of PSUM: self-loop add,
    dinv scales, relu / transpose + W2 matmul + sigmoid.

Synchronization: Tile's prepare_only consumer sync is broken (it pre-bumps
the DMASW lane sems, so auto-generated waits pass before the DMA lands), and
a single shared completion sem cannot prove an individual batch drained (the
16 SDMA engines increment it independently).  Sound scheme used here: all
SWDGE work on one queue (ring FIFO = pool order, pinned with no-sync dep
edges), grouped into windows of 2 with the completion sem alternating
between two sems; after window i+1's generation a pool barrier waits
sem[i%2] >= 16*(all instructions ever assigned to that sem) -- nothing newer
on that sem is in flight, so this proves full drain -- then bumps a fence
sem consumers wait on.  Ring occupancy stays <= 2 windows = 4096 descriptors
= the carveout from dynamic_dma_scratch_size=65536.

Host side does only index/metadata preprocessing (sorting edges, degree
counts, chunk layout, slot maps) and input re-layout (x transposed + bf16).

v2 changes vs the first working version:
  - The z / h1 tables and their AllGathers are split in half ("half-major"
    global table layout: ranges = (half, core-quad), each a contiguous
    whole-tensor collective) so the first half's AllGather overlaps the
    compute producing the second half (z phase / layer-1 epilogues).
  - x is shipped as per-dst-block contiguous [NB, 128, KT*128] tiles
    (3KB/partition DMA lines instead of 256B).
  - The merge candidate window (MK, OFF) is measured host-side from the
    actual slot-tile offsets instead of a fixed [-2,+1] heuristic.
  - DRAM zero-fill removed (unwritten table bytes are never consumed).

Measured on HW: 5.31ms (baseline of this session: 5.41ms; originally
reported 6.70ms).  GpSimd descgen remains ~80% of span.  Negative
results (do NOT retry naively): 4 SWDGE queues give zero descgen
overlap; explicit trigger counts (count=int) hang or crash the device;
coalescing count=None triggers to 1-per-4-preps regresses ~600us;
6-batch prep-ahead bursts + Shared-space tables passed once untraced
but crashed unrecoverably under profiling.
"""

import os
import numpy as np
import ml_dtypes

import concourse.bass as bass
import concourse.bacc as bacc
import concourse.tile as tile
from concourse.tile_rust import add_dep_helper
import concourse.mybir as mybir
from concourse.bass_utils import run_bass_kernel_spmd

BF16 = mybir.dt.bfloat16
FP16 = mybir.dt.float16
F32 = mybir.dt.float32
I16 = mybir.dt.int16

C = 8        # cores
P = 128      # partitions
SLOT = 32    # dst slots per chunk (chunk spans < 32 dst nodes)
DEAD = SLOT  # col_rel value marking a dead (padded) edge


def _merge_ct(b, tpr, npcp):
    # expected tile of dst block b's slots in a range region (slot ~ rho*dst)
    return int((b * 128 + 64) * (tpr * 128) / npcp) // 128


def _merge_t0(b, tpr, npcp, k, off):
    # host-measured slot-tile offsets vs the center tile lie in
    # [-off, -off+k-1]
    return min(max(_merge_ct(b, tpr, npcp) - off, 0), max(tpr - k, 0))


def _cfg_for(n_nodes, fin, hid, out_dim, ch_r, gb, mk):
    npc = n_nodes // C
    nb = -(-npc // P)
    npcp = nb * P
    kt = -(-fin // P)
    cfg = dict(
        N=n_nodes, FIN=fin, HID=hid, OUT=out_dim,
        NPC=npc, NB=nb, NPCP=npcp, KT=kt, KP=kt * P,
        RN=2 * npcp,                  # rows per gather range (2 cores)
        TBL=C * npcp,                 # allgathered table rows
        CH_R=ch_r,                    # chunks per (core, range), uniform
        GB=gb,                        # gather batch tokens
        BR=(ch_r * P) // gb,          # gather batches per range
        TPR=ch_r // 4,                # partial-sum tiles per range region
        MK=mk,                        # merge candidate tiles per (block, region)
    )
    assert cfg["BR"] * gb == ch_r * P and ch_r % 8 == 0 and gb % 128 == 0
    return cfg


# ----------------------------------------------------------------- host prep

def _preprocess(x, edge_index, W1, b1, W2, b2):
    N, FIN = x.shape
    HID = W1.shape[1]
    OUT = W2.shape[1]
    assert N % C == 0
    NPC = N // C
    NB = -(-NPC // P)
    NPCP = NB * P
    RN = 2 * NPCP

    rows = edge_index[0].astype(np.int64)
    cols = edge_index[1].astype(np.int64)

    # degree includes the self loop (reference adds it before segment_sum)
    deg = (np.bincount(cols, minlength=N) + 1).astype(np.float32)
    dinv = (1.0 / np.sqrt(deg.astype(np.float64))).astype(np.float32)

    # table row of node n in the allgathered table.  Layout is HALF-major:
    # [A-halves of cores 0..7 | B-halves of cores 0..7], where a core's
    # A-half is its first HA(=NBA*P) padded-local rows.  Each AllGather
    # half is then a plain contiguous collective, and each of the 4 gather
    # ranges (a half of a core quad) stays a contiguous RN-row region.
    NBA = (NPCP // P) // 2
    HA = NBA * P
    HB = NPCP - HA
    src_c = rows // NPC
    src_i = rows % NPC
    tbl_row = np.where(
        src_i < HA,
        src_c * HA + src_i,
        C * HA + src_c * HB + (src_i - HA),
    )
    src_range = tbl_row // RN
    idx_local = (tbl_row - src_range * RN).astype(np.int64)
    core = cols // NPC
    col_local = (cols - core * NPC).astype(np.int64)

    order = np.lexsort((col_local, src_range, core))
    core_s = core[order]
    rng_s = src_range[order]
    coll_s = col_local[order]
    idxl_s = idx_local[order]

    # chunk every (core, range) segment: break at 128 tokens or dst span 32
    bounds_all = {}
    max_chunks = 0
    seg_edges = {}
    for c in range(C):
        c_end = np.searchsorted(core_s, c + 1)
        c_start = np.searchsorted(core_s, c)
        for r in range(4):
            s0 = c_start + np.searchsorted(rng_s[c_start:c_end], r)
            s1 = c_start + np.searchsorted(rng_s[c_start:c_end], r + 1)
            seg_edges[(c, r)] = (s0, s1)
            cseg = coll_s[s0:s1]
            bounds = []
            i = 0
            n = len(cseg)
            while i < n:
                j = int(np.searchsorted(cseg, cseg[i] + SLOT, side="left"))
                j = min(j, i + P, n)
                bounds.append((i, j))
                i = j
            bounds_all[(c, r)] = bounds
            max_chunks = max(max_chunks, len(bounds))
    ch_r = max(8, ((max_chunks + 7) // 8) * 8)
    # dma_gather is limited to 1024 indices per instruction (SWDGE
    # descriptor-ring capacity; 2048 hangs the device - verified twice).
    gb = 1024

    # adaptive merge window: measure, over every (core, range, chunk), how
    # far live slots' tiles stray from their dst block's center tile
    tpr0 = ch_r // 4
    dmin, dmax = 0, 0
    for (c, r), bounds in bounds_all.items():
        s0, s1 = seg_edges[(c, r)]
        cseg = coll_s[s0:s1]
        for j, (s, e) in enumerate(bounds):
            cr = cseg[s:e] - cseg[s]
            tt = (SLOT * j + cr) // P
            ct = ((cseg[s] + cr) // P * P + 64) * (tpr0 * P) // NPCP // P
            d = tt - ct
            dmin = min(dmin, int(d.min()))
            dmax = max(dmax, int(d.max()))
    OFF = -dmin
    MK = dmax - dmin + 1
    cfg = _cfg_for(N, FIN, HID, OUT, ch_r, gb, MK)
    cfg["OFF"] = OFF
    CH_R, GB, BR, TPR = cfg["CH_R"], cfg["GB"], cfg["BR"], cfg["TPR"]
    CHUNKS = 4 * CH_R
    DUMMY = NPCP  # dummy dst row for dead slots (never matches a block row)

    # weights / tables, shared across cores
    KP = cfg["KP"]
    KT = KP // P
    xt = np.zeros((KP, C * NPCP), dtype=ml_dtypes.bfloat16)
    xtf = np.ascontiguousarray(x.T).astype(ml_dtypes.bfloat16)
    for c in range(C):
        xt[:FIN, c * NPCP:c * NPCP + NPC] = xtf[:, c * NPC:(c + 1) * NPC]
    w1 = np.zeros((KP, 64), dtype=ml_dtypes.bfloat16)
    w1[:FIN, :HID] = W1.astype(ml_dtypes.bfloat16)
    w2 = np.zeros((64, OUT), dtype=ml_dtypes.bfloat16)
    w2[:HID, :] = W2.astype(ml_dtypes.bfloat16)
    iota32 = np.tile(np.arange(SLOT, dtype=np.float32), (P, 1)).astype(ml_dtypes.bfloat16)
    iota128 = np.tile(np.arange(P, dtype=np.float32), (P, 1))
    ident = np.eye(P, dtype=np.float32).astype(ml_dtypes.bfloat16)
    b1r = np.zeros((1, 64), np.float32)
    b1r[0, :HID] = b1
    b2r = b2.reshape(1, OUT).astype(np.float32)
    has_b1 = bool(np.any(b1))
    has_b2 = bool(np.any(b2))

    in_maps = []
    for c in range(C):
        gidx = np.zeros((4 * BR, P, GB // 16), np.int16)
        colrel_tile = np.full((P, CHUNKS), float(DEAD), np.float32)
        mrel = np.full((P, NB * 4 * MK), float(2 * P), np.float32)

        for r in range(4):
            s0, s1 = seg_edges[(c, r)]
            cseg = coll_s[s0:s1]
            iseg = idxl_s[s0:s1]
            bounds = bounds_all[(c, r)]
            gtok = np.zeros((CH_R, P), np.int64)
            crel = np.full((CH_R, P), DEAD, np.int64)
            sreg = np.full(TPR * P, DUMMY, np.int64)
            for j, (s, e) in enumerate(bounds):
                L = e - s
                # sort the chunk's tokens by src table row (HBM locality);
                # crel follows the same permutation so S1 still matches.
                perm = np.argsort(iseg[s:e], kind="stable")
                gtok[j, :L] = iseg[s:e][perm]
                if L < P:
                    gtok[j, L:] = gtok[j, L - 1]
                cr_sorted = (cseg[s:e] - cseg[s])[perm]
                crel[j, :L] = cr_sorted
                cr = cseg[s:e] - cseg[s]
                # slot map: chunk j occupies slots [32j, 32j+32)
                sreg[SLOT * j:SLOT * (j + 1)][cr] = cseg[s] + cr
            # assemble per-core tensors
            colrel_tile[:, r * CH_R:(r + 1) * CH_R] = crel.T
            for bi in range(BR):
                toks = gtok[bi * (GB // P):(bi + 1) * (GB // P)].reshape(-1)
                gidx[r * BR + bi] = np.tile(
                    toks.reshape(GB // 16, 16).T, (8, 1))
            # merge one-hot columns for this range region; coverage check:
            # every live slot's tile must be inside its block's window
            live = sreg != DUMMY
            if live.any():
                sl = np.nonzero(live)[0]
                bb = sreg[sl] // 128
                tt = sl // 128
                t0s = np.array([_merge_t0(b, TPR, NPCP, MK, OFF) for b in bb])
                assert ((tt >= t0s) & (tt < t0s + MK)).all(), \
                    "merge window MK too small"
            for b in range(NB):
                t0 = _merge_t0(b, TPR, NPCP, MK, OFF)
                for k in range(MK):
                    col = b * (4 * MK) + r * MK + k
                    seg = sreg[(t0 + k) * P:(t0 + k + 1) * P]
                    mrel[:, col] = seg - 128 * b

        nb = cfg["NB"]
        dloc = np.ones(NPCP, np.float32)
        dloc[:NPC] = dinv[c * NPC:(c + 1) * NPC]
        dinv_pp = dloc.reshape(nb, P).T.copy()          # [128, NB]
        dinv2_pp = (dloc * dloc).reshape(nb, P).T.copy()
        sqdloc = np.ones(NPCP, np.float32)
        sqdloc[:NPC] = np.sqrt(deg[c * NPC:(c + 1) * NPC])

        # xtb[b, p, k, n]: per dst-block contiguous (3KB/partition DMAs)
        xtb = np.ascontiguousarray(
            xt[:, c * NPCP:(c + 1) * NPCP]
            .reshape(KT, P, NB, P).transpose(2, 1, 0, 3)
            .reshape(NB, P, KT * P))

        m = {
            "xtb": xtb,
            "w1": w1, "w2": w2, "iota32": iota32, "iota128": iota128,
            "ident": ident,
            "colrel": colrel_tile.astype(ml_dtypes.bfloat16),
            "gidx": gidx.astype(np.int16),
            "mrel": mrel,
            "dinv_pp": dinv_pp, "dinv2_pp": dinv2_pp,
            "b1f": np.tile(b1r, (P, 1)), "b2r": b2r,
            "sqd": sqdloc.reshape(1, NPCP),
            "sqd_pp": sqdloc.reshape(nb, P).T.copy(),
        }
        in_maps.append(m)

    cfg["HAS_B1"] = has_b1
    cfg["HAS_B2"] = has_b2
    return cfg, in_maps


# ------------------------------------------------------------- program build

def _build_program(cfg, phases="full"):
    NB, KT, NPCP, RN, TBL = cfg["NB"], cfg["KT"], cfg["NPCP"], cfg["RN"], cfg["TBL"]
    CH_R, GB, BR, TPR = cfg["CH_R"], cfg["GB"], cfg["BR"], cfg["TPR"]
    OUT, MK, OFF = cfg["OUT"], cfg["MK"], cfg["OFF"]
    CHUNKS = 4 * CH_R
    SPB = GB // P      # chunk slots per gather batch
    NPAIR = SPB // 8   # psum-tile pairs per batch

    nc = bacc.Bacc("TRN2", target_bir_lowering=False, debug=False,
                   num_devices=C, num_swdge_queues=1,
                   dynamic_dma_scratch_size=65536)

    xt_d = nc.dram_tensor("xtb", [NB, P, KT * P], BF16, kind="ExternalInput")
    w1_d = nc.dram_tensor("w1", [cfg["KP"], 64], BF16, kind="ExternalInput")
    w2_d = nc.dram_tensor("w2", [64, OUT], BF16, kind="ExternalInput")
    iota_d = nc.dram_tensor("iota32", [P, SLOT], BF16, kind="ExternalInput")
    iota128_d = nc.dram_tensor("iota128", [P, P], F32, kind="ExternalInput")
    ident_d = nc.dram_tensor("ident", [P, P], BF16, kind="ExternalInput")
    colrel_d = nc.dram_tensor("colrel", [P, CHUNKS], BF16, kind="ExternalInput")
    gidx_d = nc.dram_tensor("gidx", [4 * BR, P, GB // 16], I16, kind="ExternalInput")
    mrel_d = nc.dram_tensor("mrel", [P, NB * 4 * MK], F32, kind="ExternalInput")
    dinv_d = nc.dram_tensor("dinv_pp", [P, NB], F32, kind="ExternalInput")
    dinv2_d = nc.dram_tensor("dinv2_pp", [P, NB], F32, kind="ExternalInput")
    b1f_d = nc.dram_tensor("b1f", [P, 64], F32, kind="ExternalInput")
    b2_d = nc.dram_tensor("b2r", [1, OUT], F32, kind="ExternalInput")
    sqd_d = nc.dram_tensor("sqd", [1, NPCP], F32, kind="ExternalInput")
    sqdpp_d = nc.dram_tensor("sqd_pp", [P, NB], F32, kind="ExternalInput")
    out_d = nc.dram_tensor("out", [NPCP, OUT], F32, kind="ExternalOutput")
    dbg_d = None
    if phases != "full":
        dbg_d = nc.dram_tensor("dbg", [TBL, P], F32, kind="ExternalOutput")

    # ---- SWDGE window/fence machinery (see module docstring) -------------
    WIN = 2
    wsems = [nc.alloc_semaphore("sww0"), nc.alloc_semaphore("sww1")]
    fence_sem = nc.alloc_semaphore("swfence")
    sw = {
        "counts": [0, 0], "wid": 0, "items": 0, "pending": [],
        "last": None, "fences": 0, "gwids": [],
    }

    def _chain(bi):
        if sw["last"] is not None:
            add_dep_helper(bi.ins, sw["last"], sync=False,
                           reason="swdge ring order")
        sw["last"] = bi.ins
        return bi

    def _emit_barrier():
        wid, sidx, cnt = sw["pending"].pop(0)
        _chain(nc.gpsimd.wait_ge(wsems[sidx], 16 * cnt))
        _chain(nc.gpsimd.sem_inc(fence_sem, 1))
        sw["fences"] += 1
        assert sw["fences"] == wid + 1

    def _close_window():
        if sw["items"] == 0:
            return
        sidx = sw["wid"] % 2
        sw["pending"].append((sw["wid"], sidx, sw["counts"][sidx]))
        sw["wid"] += 1
        sw["items"] = 0

    def sw_sem():
        return wsems[sw["wid"] % 2]

    def sw_track(prep):
        wid = sw["wid"]
        sw["counts"][wid % 2] += 1
        sw["items"] += 1
        _chain(prep)
        _chain(nc.gpsimd.trigger_dma(count=None, queue_num=0))
        if sw["items"] == WIN:
            _close_window()
            while len(sw["pending"]) >= 2:
                _emit_barrier()
        return wid

    def sw_flush():
        _close_window()
        while sw["pending"]:
            _emit_barrier()
        return sw["fences"]

    GIXT_BUFS = 6

    with tile.TileContext(nc) as tc:
        with tc.tile_pool(name="const", bufs=1) as cpool, \
             tc.tile_pool(name="sb", bufs=2) as sb, \
             tc.tile_pool(name="msgp", bufs=4) as msgp, \
             tc.tile_pool(name="parts", bufs=4) as parts_pool, \
             tc.tile_pool(name="psum", bufs=2, space="PSUM") as pp, \
             tc.tile_pool(name="dram", bufs=1, space="DRAM") as dram:

            # ---- constants
            w1_t = cpool.tile([P, KT, 64], BF16)
            nc.sync.dma_start(
                out=w1_t[:], in_=w1_d.ap().rearrange("(k p) e -> p k e", p=P))
            w2_t = cpool.tile([64, OUT], BF16)
            nc.sync.dma_start(out=w2_t[:], in_=w2_d.ap())
            iota_t = cpool.tile([P, SLOT], BF16)
            nc.sync.dma_start(out=iota_t[:], in_=iota_d.ap())
            iota128_t = cpool.tile([P, P], F32)
            nc.sync.dma_start(out=iota128_t[:], in_=iota128_d.ap())
            ident_t = cpool.tile([P, P], BF16)
            nc.sync.dma_start(out=ident_t[:], in_=ident_d.ap())
            colrel_t = cpool.tile([P, CHUNKS], BF16)
            nc.sync.dma_start(out=colrel_t[:], in_=colrel_d.ap())
            mrel_t = cpool.tile([P, NB * 4 * MK], F32)
            nc.sync.dma_start(out=mrel_t[:], in_=mrel_d.ap())
            dinv_t = cpool.tile([P, NB], F32)
            nc.sync.dma_start(out=dinv_t[:], in_=dinv_d.ap())
            dinv2_t = cpool.tile([P, NB], F32)
            nc.sync.dma_start(out=dinv2_t[:], in_=dinv2_d.ap())
            b2_t = cpool.tile([1, OUT], F32)
            nc.sync.dma_start(out=b2_t[:], in_=b2_d.ap())
            sqd_t = None
            if cfg["HAS_B2"]:
                sqd_t = cpool.tile([1, NPCP], F32)
                nc.sync.dma_start(out=sqd_t[:], in_=sqd_d.ap())
            b1f_t = None
            sqdpp_t = None
            if cfg["HAS_B1"]:
                b1f_t = cpool.tile([P, 64], F32)
                nc.sync.dma_start(out=b1f_t[:], in_=b1f_d.ap())
                sqdpp_t = cpool.tile([P, NB], F32)
                nc.sync.dma_start(out=sqdpp_t[:], in_=sqdpp_d.ap())

            # SBUF-resident local scaled tables (self-loop add in epilogues)
            z_sb = cpool.tile([P, NB, 64], BF16)
            h1_sb = cpool.tile([P, NB, 64], BF16)



            # ---- DRAM temporaries.  The per-core local tables are split
            # into two half tiles so each half's AllGather can launch as
            # soon as that half's blocks are written (overlapping the
            # collective with remaining compute).
            NBA = NB // 2
            HA, HB = NBA * P, NPCP - NBA * P
            z_locA = dram.tile([HA, P], BF16, name="z_locA")
            z_locB = dram.tile([HB, P], BF16, name="z_locB")
            zfullA = dram.tile([C * HA, P], BF16, name="zfullA")
            zfullB = dram.tile([C * HB, P], BF16, name="zfullB")
            h1_locA = dram.tile([HA, P], BF16, name="h1_locA")
            h1_locB = dram.tile([HB, P], BF16, name="h1_locB")
            h1fullA = dram.tile([C * HA, P], BF16, name="h1fullA")
            h1fullB = dram.tile([C * HB, P], BF16, name="h1fullB")

            def loc_row(t_a, t_b, b):
                if b < NBA:
                    return t_a[:][b * P:(b + 1) * P, 0:64]
                return t_b[:][(b - NBA) * P:(b - NBA) * P + P, 0:64]

            def ag_half(loc, full):
                # half-major table: ranges 0,1 live in the A tensor,
                # ranges 2,3 in the B tensor; each AllGather is a plain
                # whole-tile contiguous collective.
                nc.gpsimd.collective_compute(
                    "AllGather", mybir.AluOpType.bypass,
                    replica_groups=[list(range(C))],
                    ins=[loc.opt()], outs=[full.opt()])

            # ---- phase Z: z_loc = dinv * (x @ W1), node-major bf16
            for b in range(NB):
                xtt = sb.tile([P, KT, P], BF16, tag="xtt")
                nc.sync.dma_start(out=xtt[:], in_=xt_d.ap()[b])
                psz = pp.tile([P, 64], F32, tag="psz")
                for k in range(KT):
                    nc.tensor.matmul(
                        out=psz[:], lhsT=xtt[:, k, :], rhs=w1_t[:, k, :],
                        start=(k == 0), stop=(k == KT - 1))
                nc.scalar.activation(
                    out=z_sb[:, b, :], in_=psz[:],
                    func=mybir.ActivationFunctionType.Copy,
                    scale=dinv_t[:, b:b + 1])
                nc.sync.dma_start(
                    out=loc_row(z_locA, z_locB, b), in_=z_sb[:, b, :])
                if b == NBA - 1:
                    ag_half(z_locA, zfullA)
            ag_half(z_locB, zfullB)

            def dump_dbg(src, rows, row_off=0):
                nc.gpsimd.dma_start(
                    out=dbg_d.ap()[row_off:row_off + rows, :], in_=src[:][:rows, :])

            # ---- aggregation layer: bi-major gathers + chunk-reduce into
            # 4 range regions [P, TPR, 64] fp16, with per-dst-block merge
            # matmuls + the layer epilogue STREAMING behind the gather rows
            # (block b is merged once tiles t0(b)..t0(b)+MK-1 of every
            # region are complete, i.e. after row (t0(b)+MK)/2 - 1).
            def merge_block(b, partials, epilogue_fn):
                t0 = _merge_t0(b, TPR, NPCP, MK, OFF)
                oh = sb.tile([P, 4 * MK, P], FP16, tag="oh")
                c0 = b * (4 * MK)
                nc.vector.tensor_tensor(
                    out=oh[:],
                    in0=iota128_t[:][:, None, :].to_broadcast([P, 4 * MK, P]),
                    in1=mrel_t[:, c0:c0 + 4 * MK][:, :, None].to_broadcast(
                        [P, 4 * MK, P]),
                    op=mybir.AluOpType.is_equal)
                psm = pp.tile([P, 64], F32, tag="mg")
                i = 0
                n_mm = 4 * MK
                for r in range(4):
                    for k in range(MK):
                        nc.tensor.matmul(
                            out=psm[:], lhsT=oh[:, r * MK + k, :],
                            rhs=partials[r][:, t0 + k, :],
                            start=(i == 0), stop=(i == n_mm - 1),
                            skip_group_check=True)
                        i += 1
                epilogue_fn(b, psm)

            def agg_layer(src_full, epilogue_fn):
                fullA, fullB = src_full
                partials = [parts_pool.tile([P, TPR, 64], FP16, tag="parts",
                                             name=f"parts_r{r}")
                            for r in range(4)]
                next_b = [0]

                def drain_ready(tiles_done):
                    while next_b[0] < NB and (
                            _merge_t0(next_b[0], TPR, NPCP, MK, OFF) + MK
                            <= tiles_done):
                        merge_block(next_b[0], partials, epilogue_fn)
                        next_b[0] += 1

                # front-load A-range (zfullA/h1fullA) batches so the
                # first triggers never wait on the B-half AllGather
                FRONT = min(8, BR)
                batch_list = []
                for bi in range(FRONT):
                    for r in (0, 1):
                        batch_list.append((bi, r))
                for bi in range(BR):
                    rs = (2, 3) if bi < FRONT else (0, 1, 2, 3)
                    for r in rs:
                        batch_list.append((bi, r))
                assert len(batch_list) == 4 * BR
                td = [0, 0, 0, 0]
                for bi, r in batch_list:
                    if True:
                        bidx = r * BR + bi
                        g = len(sw["gwids"])
                        gixt = sb.tile([P, GB // 16], I16, tag="gixt",
                                       bufs=GIXT_BUFS)
                        gl = nc.sync.dma_start(out=gixt[:], in_=gidx_d.ap()[bidx])
                        if g >= GIXT_BUFS:
                            # deferred gather reads its idx tile when the
                            # trigger fires; the fence covering the buffer's
                            # previous user implies that trigger fired
                            gl._wait_ge(fence_sem, sw["gwids"][g - GIXT_BUFS] + 1)
                        msg = msgp.tile([P, SPB, P], BF16, tag="msg")
                        src_t = fullA if r < 2 else fullB
                        rr = r % 2
                        prep = nc.gpsimd.dma_gather(
                            out_ap=msg[:],
                            in_ap=src_t[:][rr * RN:(rr + 1) * RN, :],
                            idxs_ap=gixt[:],
                            num_idxs=GB, num_idxs_reg=GB, elem_size=P,
                            prepare_only=True, sem=sw_sem(), queue_num=0)
                        gwid = sw_track(prep)
                        sw["gwids"].append(gwid)
                        s1t = sb.tile([P, SPB, SLOT], BF16, tag="s1t")
                        cb = bidx * SPB
                        nc.vector.tensor_tensor(
                            out=s1t[:],
                            in0=iota_t[:][:, None, :].to_broadcast([P, SPB, SLOT]),
                            in1=colrel_t[:, cb:cb + SPB][:, :, None].to_broadcast(
                                [P, SPB, SLOT]),
                            op=mybir.AluOpType.is_equal)
                        for pair in range(NPAIR):
                            ps_e = pp.tile([P, 64], F32, tag="pse")
                            ps_o = pp.tile([P, 64], F32, tag="pso")
                            ps = [ps_e, ps_o]
                            for jj in range(8):
                                cl = pair * 8 + jj
                                q4 = jj % 4
                                mm = nc.tensor.matmul(
                                    out=ps[jj // 4][SLOT * q4:SLOT * (q4 + 1), :],
                                    lhsT=s1t[:, cl, :],
                                    rhs=msg[:, cl, 0:64],
                                    start=True, stop=True,
                                    tile_position=(0, SLOT * q4),
                                    skip_group_check=True)
                                mm._wait_ge(fence_sem, gwid + 1)
                            trb = (bi * NPAIR + pair) * 2
                            nc.vector.tensor_copy(
                                out=partials[r][:, trb, :], in_=ps[0][:])
                            nc.scalar.copy(
                                out=partials[r][:, trb + 1, :], in_=ps[1][:])
                    td[r] += 2 * NPAIR
                    drain_ready(max(0, min(td) - 2))
                sw_flush()
                drain_ready(TPR)
                assert next_b[0] == NB

            order = ["z", "l1", "full"]
            lvl = order.index(phases) if phases in order else 2

            # L1 epilogue: h1 = relu(dinv^2*(agg + z_self) [+ dinv*b1])
            def epilogue1(b, psm):
                st = sb.tile([P, 64], F32, tag="st")
                nc.vector.tensor_tensor(
                    out=st[:], in0=psm[:], in1=z_sb[:, b, :],
                    op=mybir.AluOpType.add)
                if cfg["HAS_B1"]:
                    tmp = sb.tile([P, 64], F32, tag="tmpb")
                    nc.vector.tensor_scalar_mul(
                        tmp[:], b1f_t[:], sqdpp_t[:, b:b + 1])
                    nc.vector.tensor_tensor(
                        out=st[:], in0=st[:], in1=tmp[:], op=mybir.AluOpType.add)
                nc.scalar.activation(
                    out=h1_sb[:, b, :], in_=st[:],
                    func=mybir.ActivationFunctionType.Relu,
                    scale=dinv2_t[:, b:b + 1])
                nc.sync.dma_start(
                    out=loc_row(h1_locA, h1_locB, b), in_=h1_sb[:, b, :])
                if b == NBA - 1:
                    ag_half(h1_locA, h1fullA)

            # L2 epilogue: out = sigmoid(dinv * ((agg2 + h1_self) @ W2) [+ b2])
            def epilogue2(b, psm):
                st = sb.tile([P, 64], BF16, tag="st2")
                nc.vector.tensor_tensor(
                    out=st[:], in0=psm[:], in1=h1_sb[:, b, :],
                    op=mybir.AluOpType.add)
                tp = pp.tile([64, P], BF16, tag="psz")
                nc.tensor.transpose(out=tp[:], in_=st[:], identity=ident_t[:])
                zt = sb.tile([64, P], BF16, tag="zt")
                nc.scalar.copy(out=zt[:], in_=tp[:])
                ps3 = pp.tile([P, 64], F32, tag="pso")
                nc.tensor.matmul(
                    out=ps3[:, 0:OUT], lhsT=zt[:], rhs=w2_t[:],
                    start=True, stop=not cfg["HAS_B2"],
                    skip_group_check=True)
                if cfg["HAS_B2"]:
                    nc.tensor.matmul(
                        out=ps3[:, 0:OUT], lhsT=sqd_t[:, b * P:(b + 1) * P],
                        rhs=b2_t[:], start=False, stop=True,
                        skip_group_check=True)
                ot = sb.tile([P, OUT], F32, tag="ot")
                nc.scalar.activation(
                    out=ot[:], in_=ps3[:, 0:OUT],
                    func=mybir.ActivationFunctionType.Sigmoid,
                    scale=dinv_t[:, b:b + 1])
                nc.sync.dma_start(
                    out=out_d.ap()[b * P:(b + 1) * P, :], in_=ot[:])

            # ================= layer 1 =================
            if lvl >= 1:
                agg_layer((zfullA, zfullB), epilogue1)
            if phases == "z":
                dump_dbg(zfullA, C * HA)
                dump_dbg(zfullB, C * HB, row_off=C * HA)
            if phases == "l1":
                dump_dbg(h1_locA, HA)
                dump_dbg(h1_locB, HB, row_off=HA)

            if lvl >= 2:
                ag_half(h1_locB, h1fullB)

                # ================= layer 2 =================
                agg_layer((h1fullA, h1fullB), epilogue2)

    nc.compile()
    return nc


_PROGRAM_CACHE = {}
LAST_EXEC_NS = None
LAST_TRACE = None


def _get_program(cfg):
    key = tuple(sorted((k, v) for k, v in cfg.items()))
    if key not in _PROGRAM_CACHE:
        _PROGRAM_CACHE[key] = _build_program(cfg)
    return _PROGRAM_CACHE[key]


def kernel(x, edge_index, W1, b1, W2, b2):
    x = np.asarray(x, np.float32)
    edge_index = np.asarray(edge_index)
    W1 = np.asarray(W1, np.float32)
    b1 = np.asarray(b1, np.float32)
    W2 = np.asarray(W2, np.float32)
    b2 = np.asarray(b2, np.float32)

    cfg, in_maps = _preprocess(x, edge_index, W1, b1, W2, b2)
    nc = _get_program(cfg)
    trace = bool(os.environ.get("KERNEL_TRACE"))
    res = run_bass_kernel_spmd(nc, in_maps, core_ids=list(range(C)), trace=trace)
    global LAST_EXEC_NS, LAST_TRACE
    if res.exec_time_ns:
        LAST_EXEC_NS = res.exec_time_ns
        LAST_TRACE = res
    NPC, OUT = cfg["NPC"], cfg["OUT"]
    out = np.empty((cfg["N"], OUT), np.float32)
    for c in range(C):
        out[c * NPC:(c + 1) * NPC] = res.results[c]["out"][:NPC]
    return out

